# revision 4
# baseline (speedup 1.0000x reference)
"""Block-circulant linear layer (y = x @ W^T + bias, W built from 64x64
circulant blocks) on 8 Trainium2 NeuronCores.

Math: per output block j, input block i: y[t,j] = sum_i circ(c[j,i]) @ x[t,i].
Via the convolution theorem this is, for each rfft bin k:
    Yhat[t,j,k] = sum_i Chat[j,i,k] * Xhat[t,i,k]   (complex)
i.e. 33 independent complex [64 x 64] matmuls over the block index, batched
over tokens. The host does the cheap O(T*F*logB) DFTs + layout packing; the
device does the dominant compute — the per-frequency complex matmuls — packed
as real [128x128] @ [128x512] matmuls.

Real/complex packing (per frequency k, contraction over rows r):
    rhs rows r:   [Xr_i (64) ; Xi_i (64)],  cols = tokens
    lhsT[i,    j] =  Cr[j,i]    lhsT[i,    64+j] = Ci[j,i]
    lhsT[64+i, j] = -Ci[j,i]    lhsT[64+i, 64+j] = Cr[j,i]
    out rows:     [Yr_j (64) ; Yi_j (64)]
Bins k=0 and k=32 are purely real (real input DFT), so they share one tile
(kt=0) with a block-diagonal lhsT; kt=1..31 carry bin k = kt.

Precision: X ships as fp8 e3m4 (1 byte) with a per-bin scale sx[k] =
absmax/15 folded into C (C' = Chat * sx, fp16).  The PE upconverts both
operands to fp22 and accumulates fp32, so the mixed fp16 x fp8 matmul is
exact given the quantized inputs; measured end-to-end max rel err ~1.4e-2
(gate 2e-2).  Y returns as fp16 (~3e-4 additional).

Sharding: by frequency tile (4 kt per core), NOT by tokens — the per-core
weight slice is 131KB instead of a replicated 1.05MB, and the kernel is
DMA-engine-bound: the 16 DMA engines sustain ~25 B/ns each (~410 B/ns/core
aggregate), so bytes moved is the whole game: 2.23MB in + 4.19MB out per
core.  Loads ride the SP queue whose FIFO then sequences the even stores
strictly after them (mixed-direction HBM runs ~30% slower); odd stores drain
in parallel on the ACT queue.  Compute pipelines over 8 subchunks
(matmul -> DVE/ACT cast); GpSimd has no PSUM port so only those two engines
can cast.
"""

import numpy as np
import ml_dtypes

_B = 64          # circulant block size
_NBLK = 64       # input/output blocks (4096/64)
_NK = 33         # rfft bins of a 64-point real signal
_NKT = 32        # packed frequency tiles (k0+k32 share tile 0)
_NCORES = 8
_KTC = _NKT // _NCORES   # 4 frequency tiles per core
_T = 4096        # tokens = 2*2048
_F = 4096

_GL = 4           # token chunks per core for LOADS (4KB rows at e3m4)
_TCL = _T // _GL  # 1024 tokens per load chunk
_GS = 8           # subchunks for compute/stores (4KB rows, fine store pipeline)
_TCS = _T // _GS  # 512 tokens per store subchunk

_E3 = ml_dtypes.float8_e3m4
_E3_TOP = 15.0    # scale X bins so absmax maps here (e3m4 max = 15.5)

_CACHE = {}


def _fold_scales(fc):
    """fc: [J, I, 33] complex64 -> (fc_scaled, sx[33]) with per-bin absmax
    scales to divide X by; the scale is multiplied into C."""
    return fc  # scaling handled in _pack_all


def _build_cmat(fc_s):
    """fc_s: [J, I, 33] complex64 (already bin-scaled) -> lhsT [128, NKT*128] fp16."""
    Cr, Ci = fc_s.real, fc_s.imag
    cm = np.zeros((_NKT, 128, 128), np.float32)  # [kt, row, col]
    cm[0, 0:64, 0:64] = Cr[:, :, 0].T
    cm[0, 64:128, 64:128] = Cr[:, :, 32].T
    for k in range(1, 32):
        cm[k, 0:64, 0:64] = Cr[:, :, k].T
        cm[k, 64:128, 0:64] = -Ci[:, :, k].T
        cm[k, 0:64, 64:128] = Ci[:, :, k].T
        cm[k, 64:128, 64:128] = Cr[:, :, k].T
    out = np.ascontiguousarray(cm.transpose(1, 0, 2)).reshape(128, _NKT * 128)
    return out.astype(np.float16)


def _pack_all(x, c):
    """-> (XKf [NKT,128,T] e3m4, cmat [128, NKT*128] fp16, sx[33])."""
    xb = np.asarray(x, np.float32).reshape(_T, _NBLK, _B)
    fx = np.fft.rfft(xb, axis=-1)            # [T, I, 33] complex64
    fc = np.fft.rfft(np.asarray(c, np.float32), axis=-1)  # [J, I, 33]
    R = np.ascontiguousarray(fx.real.transpose(2, 1, 0))   # [33, I, T]
    Im = np.ascontiguousarray(fx.imag.transpose(2, 1, 0))
    # per-bin scale: absmax over (t, i) of both components
    sx = np.maximum(np.abs(R).max(axis=(1, 2)), np.abs(Im).max(axis=(1, 2)))
    sx = np.where(sx > 0, sx, 1.0).astype(np.float32) / _E3_TOP   # [33]
    R /= sx[:, None, None]
    Im /= sx[:, None, None]
    XKf = np.empty((_NKT, 128, _T), _E3)
    XKf[0, 0:64] = R[0].astype(_E3)
    XKf[0, 64:128] = R[32].astype(_E3)
    XKf[1:32, 0:64] = R[1:32].astype(_E3)
    XKf[1:32, 64:128] = Im[1:32].astype(_E3)
    cmat = _build_cmat(fc * sx[None, None, :])
    return XKf, cmat


def _unpack_y(YKf, bias):
    """YKf: [NKT, 128, T] fp16 device output -> y [2, 2048, 4096] float32."""
    re = np.zeros((_NK, _NBLK, _T), np.float32)
    im = np.zeros((_NK, _NBLK, _T), np.float32)
    re[0] = YKf[0, 0:64]
    re[32] = YKf[0, 64:128]
    re[1:32] = YKf[1:32, 0:64]
    im[1:32] = YKf[1:32, 64:128]
    Yf = (re + 1j * im).transpose(2, 1, 0)   # [T, J, 33]
    yb = np.fft.irfft(Yf, n=_B, axis=-1).astype(np.float32)  # [T, J, B]
    y = yb.reshape(_T, _F) + np.asarray(bias, np.float32)
    return np.ascontiguousarray(y.reshape(2, _T // 2, _F))


def _build_device():
    import concourse.bacc as bacc
    import concourse.mybir as mybir
    import concourse.tile as tile

    f32 = mybir.dt.float32
    xdt = mybir.dt.float8e3
    cdt = mybir.dt.float16
    outdt = mybir.dt.float16
    nc = bacc.Bacc("TRN2", target_bir_lowering=False, debug=False)
    _CMW = _KTC * 128
    cw = nc.dram_tensor("cw", [128, _CMW], cdt, kind="ExternalInput")
    xk = nc.dram_tensor("xk", [_GL, 128, _KTC * _TCL], xdt, kind="ExternalInput")
    yk = nc.dram_tensor("yk", [_GS, 128, _KTC * _TCS], outdt, kind="ExternalOutput")

    with tile.TileContext(nc) as tc:
        with (
            tc.tile_pool(name="cpool", bufs=1) as cpool,
            tc.tile_pool(name="xpool", bufs=1) as xpool,
            tc.tile_pool(name="ypool", bufs=1) as ypool,
            tc.tile_pool(name="pp", bufs=3, space="PSUM") as pp,
            tc.tile_pool(name="wpp", bufs=1, space="PSUM") as wpp,
        ):
            # all loads issued upfront on the SP ring; its FIFO sequences the
            # even stores strictly after them.  Distinct buffers so no load
            # waits on anything.
            ct = cpool.tile([128, _CMW], cdt, tag="cw", name="cw")
            nc.sync.dma_start(out=ct[:], in_=cw[:, :])
            xts = []
            for g in range(_GL):
                xt = xpool.tile([128, _KTC * _TCL], xdt, tag=f"x{g}", name=f"x{g}")
                nc.sync.dma_start(out=xt[:], in_=xk[g])
                xts.append(xt)
            # PE warmup on zeroed tiles while the first loads are in flight:
            # ~3us of continuous matmul work ramps the PE p-state to 2.4GHz
            # before the real stream starts (the p-state decays on idle gaps,
            # and a cold PE runs matmuls ~2x slower).
            wlhs = cpool.tile([128, 128], cdt, tag="wlhs", name="wlhs")
            wrhs = cpool.tile([128, 512], xdt, tag="wrhs", name="wrhs")
            nc.gpsimd.memset(wlhs[:], 0.0)
            nc.gpsimd.memset(wrhs[:], 0.0)
            wps = wpp.tile([128, 512], f32, name="wps")
            for _w in range(8):
                nc.tensor.matmul(
                    wps[:], lhsT=wlhs[:], rhs=wrhs[:], start=True, stop=True
                )
            # compute/store over 8 subchunks of 512 tokens; subchunk s reads
            # from load chunk s//2 at token offset (s%2)*512
            for s in range(_GS):
                xt = xts[s * _TCS // _TCL]
                toff = (s * _TCS) % _TCL
                # distinct buffer per subchunk: casts never wait store drains
                yt = ypool.tile([128, _KTC * _TCS], outdt, tag=f"y{s}", name=f"y{s}")
                for h in range(_KTC // 2):
                    # 2-bank PSUM tile, two matmuls, one wide cast
                    ps = pp.tile([128, 2 * _TCS], f32)
                    for jj in range(2):
                        kt = h * 2 + jj
                        nc.tensor.matmul(
                            ps[:, jj * _TCS:(jj + 1) * _TCS],
                            lhsT=ct[:, kt * 128:(kt + 1) * 128],
                            rhs=xt[:, kt * _TCL + toff:kt * _TCL + toff + _TCS],
                            start=True,
                            stop=True,
                        )
                    dst = yt[:, h * 2 * _TCS:(h + 1) * 2 * _TCS]
                    # split casts across DVE and ACT (only engines with a
                    # PSUM read port)
                    if h == 0:
                        nc.vector.tensor_copy(dst, ps[:])
                    else:
                        nc.scalar.copy(dst, ps[:])
                # even stores ride the SP queue (its FIFO sequences them after
                # all loads — mixed-direction HBM runs ~30% slower); odd
                # stores ride ACT, issued late enough that loads are done.
                # Two store queues drain the tail in parallel.
                if s % 2 == 0:
                    nc.sync.dma_start(out=yk[s], in_=yt[:])
                else:
                    nc.scalar.dma_start(out=yk[s], in_=yt[:])
    nc.compile()
    return nc


def _execute(in_maps, **kwargs):
    from concourse.bass_utils import run_bass_kernel_spmd

    if "nc" not in _CACHE:
        _CACHE["nc"] = _build_device()
    return run_bass_kernel_spmd(
        _CACHE["nc"], in_maps, core_ids=list(range(_NCORES)), **kwargs
    )


def _make_in_maps(x, c):
    XKf, cmd = _pack_all(x, c)
    maps = []
    for m in range(_NCORES):
        s = XKf[m * _KTC:(m + 1) * _KTC]           # [KTC, 128, T] e3m4
        s = s.reshape(_KTC, 128, _GL, _TCL)        # [kt, p, g, t]
        xkm = np.ascontiguousarray(
            s.transpose(2, 1, 0, 3).reshape(_GL, 128, _KTC * _TCL)
        )
        cmm = np.ascontiguousarray(cmd[:, m * _KTC * 128:(m + 1) * _KTC * 128])
        maps.append({"xk": xkm, "cw": cmm})
    return maps


def _gather_yk(results):
    """Per-core yk [GS, 128, KTC*TCS] -> full [NKT, 128, T]."""
    per_core = []
    for r in results:
        ykm = np.asarray(r["yk"]).reshape(_GS, 128, _KTC, _TCS)
        per_core.append(
            ykm.transpose(2, 1, 0, 3).reshape(_KTC, 128, _T)
        )
    return np.concatenate(per_core, axis=0)


def kernel(x, c, bias, **_kwargs):
    in_maps = _make_in_maps(x, c)
    bkr = _execute(in_maps)
    return _unpack_y(_gather_yk(bkr.results), bias)


# revision 7
# speedup vs baseline: 1.0498x; 1.0498x over previous
"""Block-circulant linear layer (y = x @ W^T + bias, W built from 64x64
circulant blocks) on 8 Trainium2 NeuronCores.

Math: per output block j, input block i: y[t,j] = sum_i circ(c[j,i]) @ x[t,i].
Via the convolution theorem this is, for each rfft bin k:
    Yhat[t,j,k] = sum_i Chat[j,i,k] * Xhat[t,i,k]   (complex)
i.e. 33 independent complex [64 x 64] matmuls over the block index, batched
over tokens. The host does the cheap O(T*F*logB) DFTs + layout packing; the
device does the dominant compute — the per-frequency complex matmuls — packed
as real [128x128] @ [128x512] matmuls.

Real/complex packing (per frequency k, contraction over rows r):
    rhs rows r:   [Xr_i (64) ; Xi_i (64)],  cols = tokens
    lhsT[i,    j] =  Cr[j,i]    lhsT[i,    64+j] = Ci[j,i]
    lhsT[64+i, j] = -Ci[j,i]    lhsT[64+i, 64+j] = Cr[j,i]
    out rows:     [Yr_j (64) ; Yi_j (64)]
Bins k=0 and k=32 are purely real (real input DFT), so they share one tile
(kt=0) with a block-diagonal lhsT; kt=1..31 carry bin k = kt.

Precision: X ships as fp8 e3m4 (1 byte) with a per-bin scale sx[k] =
absmax/15 folded into C (C' = Chat * sx, fp16).  The PE upconverts both
operands to fp22 and accumulates fp32, so the mixed fp16 x fp8 matmul is
exact given the quantized inputs; measured end-to-end max rel err ~1.4e-2
(gate 2e-2).  Y returns as fp16 (~3e-4 additional).

Sharding: by frequency tile (4 kt per core), NOT by tokens — the per-core
weight slice is 131KB instead of a replicated 1.05MB, and the kernel is
DMA-engine-bound: the 16 DMA engines sustain ~25 B/ns each (~410 B/ns/core
aggregate), so bytes moved is the whole game: 2.23MB in + 4.19MB out per
core.  Loads ride the SP queue whose FIFO then sequences the even stores
strictly after them (mixed-direction HBM runs ~30% slower); odd stores drain
in parallel on the ACT queue.  Compute pipelines over 8 subchunks
(matmul -> DVE/ACT cast); GpSimd has no PSUM port so only those two engines
can cast.
"""

import numpy as np
import ml_dtypes

_B = 64          # circulant block size
_NBLK = 64       # input/output blocks (4096/64)
_NK = 33         # rfft bins of a 64-point real signal
_NKT = 32        # packed frequency tiles (k0+k32 share tile 0)
_NCORES = 8
_KTC = _NKT // _NCORES   # 4 frequency tiles per core
_T = 4096        # tokens = 2*2048
_F = 4096

_GL = 8           # token chunks per core for LOADS (2KB rows at e3m4; fine
                  # granularity starts the PE stream ~3us earlier)
_TCL = _T // _GL  # 512 tokens per load chunk
_GS = 8           # subchunks for compute/stores (4KB rows, fine store pipeline)
_TCS = _T // _GS  # 512 tokens per store subchunk

_E3 = ml_dtypes.float8_e3m4
_E3_TOP = 15.0    # scale X bins so absmax maps here (e3m4 max = 15.5)

_CACHE = {}


def _fold_scales(fc):
    """fc: [J, I, 33] complex64 -> (fc_scaled, sx[33]) with per-bin absmax
    scales to divide X by; the scale is multiplied into C."""
    return fc  # scaling handled in _pack_all


def _build_cmat(fc_s):
    """fc_s: [J, I, 33] complex64 (already bin-scaled) -> lhsT [128, NKT*128] fp16."""
    Cr, Ci = fc_s.real, fc_s.imag
    cm = np.zeros((_NKT, 128, 128), np.float32)  # [kt, row, col]
    cm[0, 0:64, 0:64] = Cr[:, :, 0].T
    cm[0, 64:128, 64:128] = Cr[:, :, 32].T
    for k in range(1, 32):
        cm[k, 0:64, 0:64] = Cr[:, :, k].T
        cm[k, 64:128, 0:64] = -Ci[:, :, k].T
        cm[k, 0:64, 64:128] = Ci[:, :, k].T
        cm[k, 64:128, 64:128] = Cr[:, :, k].T
    out = np.ascontiguousarray(cm.transpose(1, 0, 2)).reshape(128, _NKT * 128)
    return out.astype(np.float16)


def _pack_all(x, c):
    """-> (XKf [NKT,128,T] e3m4, cmat [128, NKT*128] fp16, sx[33])."""
    xb = np.asarray(x, np.float32).reshape(_T, _NBLK, _B)
    fx = np.fft.rfft(xb, axis=-1)            # [T, I, 33] complex64
    fc = np.fft.rfft(np.asarray(c, np.float32), axis=-1)  # [J, I, 33]
    R = np.ascontiguousarray(fx.real.transpose(2, 1, 0))   # [33, I, T]
    Im = np.ascontiguousarray(fx.imag.transpose(2, 1, 0))
    # per-bin scale: absmax over (t, i) of both components
    sx = np.maximum(np.abs(R).max(axis=(1, 2)), np.abs(Im).max(axis=(1, 2)))
    sx = np.where(sx > 0, sx, 1.0).astype(np.float32) / _E3_TOP   # [33]
    R /= sx[:, None, None]
    Im /= sx[:, None, None]
    XKf = np.empty((_NKT, 128, _T), _E3)
    XKf[0, 0:64] = R[0].astype(_E3)
    XKf[0, 64:128] = R[32].astype(_E3)
    XKf[1:32, 0:64] = R[1:32].astype(_E3)
    XKf[1:32, 64:128] = Im[1:32].astype(_E3)
    cmat = _build_cmat(fc * sx[None, None, :])
    return XKf, cmat


def _unpack_y(YKf, bias):
    """YKf: [NKT, 128, T] fp16 device output -> y [2, 2048, 4096] float32."""
    re = np.zeros((_NK, _NBLK, _T), np.float32)
    im = np.zeros((_NK, _NBLK, _T), np.float32)
    re[0] = YKf[0, 0:64]
    re[32] = YKf[0, 64:128]
    re[1:32] = YKf[1:32, 0:64]
    im[1:32] = YKf[1:32, 64:128]
    Yf = (re + 1j * im).transpose(2, 1, 0)   # [T, J, 33]
    yb = np.fft.irfft(Yf, n=_B, axis=-1).astype(np.float32)  # [T, J, B]
    y = yb.reshape(_T, _F) + np.asarray(bias, np.float32)
    return np.ascontiguousarray(y.reshape(2, _T // 2, _F))


def _build_device():
    import concourse.bacc as bacc
    import concourse.mybir as mybir
    import concourse.tile as tile

    f32 = mybir.dt.float32
    xdt = mybir.dt.float8e3
    cdt = mybir.dt.float16
    outdt = mybir.dt.float16
    nc = bacc.Bacc("TRN2", target_bir_lowering=False, debug=False)
    _CMW = _KTC * 128
    cw = nc.dram_tensor("cw", [128, _CMW], cdt, kind="ExternalInput")
    xk = nc.dram_tensor("xk", [_GL, 128, _KTC * _TCL], xdt, kind="ExternalInput")
    yk = nc.dram_tensor("yk", [_GS, 128, _KTC * _TCS], outdt, kind="ExternalOutput")

    with tile.TileContext(nc) as tc:
        with (
            tc.tile_pool(name="cpool", bufs=1) as cpool,
            tc.tile_pool(name="xpool", bufs=1) as xpool,
            tc.tile_pool(name="ypool", bufs=1) as ypool,
            tc.tile_pool(name="pp", bufs=3, space="PSUM") as pp,
            tc.tile_pool(name="wpp", bufs=1, space="PSUM") as wpp,
        ):
            # all loads issued upfront on the SP ring; its FIFO sequences the
            # even stores strictly after them.  Distinct buffers so no load
            # waits on anything.
            ct = cpool.tile([128, _CMW], cdt, tag="cw", name="cw")
            nc.sync.dma_start(out=ct[:], in_=cw[:, :])
            xts = []
            for g in range(_GL):
                xt = xpool.tile([128, _KTC * _TCL], xdt, tag=f"x{g}", name=f"x{g}")
                nc.sync.dma_start(out=xt[:], in_=xk[g])
                xts.append(xt)
            # PE warmup on zeroed tiles while the first loads are in flight:
            # ~3us of continuous matmul work ramps the PE p-state to 2.4GHz
            # before the real stream starts (the p-state decays on idle gaps,
            # and a cold PE runs matmuls ~2x slower).
            wlhs = cpool.tile([128, 128], cdt, tag="wlhs", name="wlhs")
            wrhs = cpool.tile([128, 512], xdt, tag="wrhs", name="wrhs")
            nc.gpsimd.memset(wlhs[:], 0.0)
            nc.gpsimd.memset(wrhs[:], 0.0)
            wps = wpp.tile([128, 512], f32, name="wps")
            for _w in range(12):
                nc.tensor.matmul(
                    wps[:], lhsT=wlhs[:], rhs=wrhs[:], start=True, stop=True
                )
            # compute/store over 8 subchunks of 512 tokens; subchunk s reads
            # from load chunk s//2 at token offset (s%2)*512
            for s in range(_GS):
                xt = xts[s * _TCS // _TCL]
                toff = (s * _TCS) % _TCL
                # distinct buffer per subchunk: casts never wait store drains
                yt = ypool.tile([128, _KTC * _TCS], outdt, tag=f"y{s}", name=f"y{s}")
                for h in range(_KTC // 2):
                    # 2-bank PSUM tile, two matmuls, one wide cast
                    ps = pp.tile([128, 2 * _TCS], f32)
                    for jj in range(2):
                        kt = h * 2 + jj
                        nc.tensor.matmul(
                            ps[:, jj * _TCS:(jj + 1) * _TCS],
                            lhsT=ct[:, kt * 128:(kt + 1) * 128],
                            rhs=xt[:, kt * _TCL + toff:kt * _TCL + toff + _TCS],
                            start=True,
                            stop=True,
                        )
                    dst = yt[:, h * 2 * _TCS:(h + 1) * 2 * _TCS]
                    # split casts across DVE and ACT (only engines with a
                    # PSUM read port)
                    if h == 0:
                        nc.vector.tensor_copy(dst, ps[:])
                    else:
                        nc.scalar.copy(dst, ps[:])
                # all stores ride the SP queue: its FIFO sequences them after
                # all loads (mixed-direction HBM runs ~30% slower), and a
                # single queue already spreads across all 16 DMA engines.
                nc.sync.dma_start(out=yk[s], in_=yt[:])
    nc.compile()
    return nc


def _execute(in_maps, **kwargs):
    from concourse.bass_utils import run_bass_kernel_spmd

    if "nc" not in _CACHE:
        _CACHE["nc"] = _build_device()
    return run_bass_kernel_spmd(
        _CACHE["nc"], in_maps, core_ids=list(range(_NCORES)), **kwargs
    )


def _make_in_maps(x, c):
    XKf, cmd = _pack_all(x, c)
    maps = []
    for m in range(_NCORES):
        s = XKf[m * _KTC:(m + 1) * _KTC]           # [KTC, 128, T] e3m4
        s = s.reshape(_KTC, 128, _GL, _TCL)        # [kt, p, g, t]
        xkm = np.ascontiguousarray(
            s.transpose(2, 1, 0, 3).reshape(_GL, 128, _KTC * _TCL)
        )
        cmm = np.ascontiguousarray(cmd[:, m * _KTC * 128:(m + 1) * _KTC * 128])
        maps.append({"xk": xkm, "cw": cmm})
    return maps


def _gather_yk(results):
    """Per-core yk [GS, 128, KTC*TCS] -> full [NKT, 128, T]."""
    per_core = []
    for r in results:
        ykm = np.asarray(r["yk"]).reshape(_GS, 128, _KTC, _TCS)
        per_core.append(
            ykm.transpose(2, 1, 0, 3).reshape(_KTC, 128, _T)
        )
    return np.concatenate(per_core, axis=0)


def kernel(x, c, bias, **_kwargs):
    in_maps = _make_in_maps(x, c)
    bkr = _execute(in_maps)
    return _unpack_y(_gather_yk(bkr.results), bias)
